# revision 25
# baseline (speedup 1.0000x reference)
"""Causal attention (weight-normed QKV proj + strictly-causal softmax) on 8 trn2 cores.

Sharding: data-parallel over batch (16 batches -> 2 per core). Each core runs an
identical NEFF (SPMD, no collectives) computing its 2 batches end-to-end.

Device-side formulation is fully "transposed" so no on-device transposes are needed:
  inputs are channels-first [C, S]  (S = H*W = 1024 spatial positions)
  Q^T, K^T computed as [C_head, S] via matmul(lhsT=W_T chunk, rhs=X)
  V computed as [S, C] via matmul(lhsT=X chunk, rhs=W_T)  (+ an appended ones
  column per head so the attention row-sums fall out of the AV matmul for free)
  S^T[k,q] = K_blk @ Q^T   (contract d=64; two heads packed in the PE array via
  partition offsets 0/64)
  P^T = exp(S^T/8) (no row-max subtraction needed: |S/8| <= ~30), with a
  strictly-upper 0/1 mask multiplied onto each diagonal block
  O^T[d,q] (+ sums row 64) = V_ext accumulate over k-blocks, causal blocks skipped
  The final row-sum division (0.07% of FLOPs) happens on the host in
  postprocess(); the q=0 column (start_mask) falls out as 0/0 -> 0 there.
  Projection work is interleaved into the attention stream at emission time to
  keep the PE's HAM clock-gate warm.
"""

import os
from contextlib import ExitStack
from itertools import chain as _chain_mod

import ml_dtypes
import numpy as np

import concourse.bass as bass
import concourse.mybir as mybir
import concourse.tile as tile
from concourse import bacc
from concourse.bass_utils import run_bass_kernel_spmd

# Problem constants (hardcoded per contest contract).
B, CQ, CK, CH, NH, H, W = 16, 512, 512, 512, 8, 32, 32
S = H * W            # 1024
DH = CH // NH        # 64
NCORES = 8
BL = B // NCORES     # 2 batches per core
C = 512
KB = S // 128        # 8 k-blocks
QCN = S // 512       # 2 q-chunks of 512

f32 = mybir.dt.float32

# Matmul dtype knob: "bf16" | "f32" | "f32r"
MM_MODE = os.environ.get("KERNEL_MM_MODE", "bf16")


def _mm_dt():
    return {"bf16": mybir.dt.bfloat16, "f32": f32, "f32r": f32}[MM_MODE]


def _mm_np():
    return {"bf16": ml_dtypes.bfloat16, "f32": np.float32, "f32r": np.float32}[MM_MODE]


def _mm_cast(ap):
    # view an f32 AP as f32r for matmul operands in f32r mode
    if MM_MODE == "f32r":
        return ap.bitcast(mybir.dt.float32r)
    return ap


def _chain(gens):
    for g in gens:
        yield from g


def build_nc(has_bv: bool):
    mm_dt = _mm_dt()
    mm_np = _mm_np()
    nc = bacc.Bacc("TRN2", target_bir_lowering=False, debug=False,
                   num_devices=NCORES)

    xq = nc.dram_tensor("xq", [BL, C, S], mm_dt, kind="ExternalInput").ap()
    xk = nc.dram_tensor("xk", [BL, C, S], mm_dt, kind="ExternalInput").ap()
    wqt = nc.dram_tensor("wqt", [C, C], mm_dt, kind="ExternalInput").ap()
    wkt = nc.dram_tensor("wkt", [C, C], mm_dt, kind="ExternalInput").ap()
    wvt = nc.dram_tensor("wvt", [C, C], mm_dt, kind="ExternalInput").ap()
    bqp = nc.dram_tensor("bqp", [128, 4], f32, kind="ExternalInput").ap()
    bkp = nc.dram_tensor("bkp", [128, 4], f32, kind="ExternalInput").ap()
    bvr = nc.dram_tensor("bvr", [1, C], f32, kind="ExternalInput").ap()
    out = nc.dram_tensor("out", [BL, NH, QCN, 65, 512], f32, kind="ExternalOutput").ap()

    # strictly-upper-triangular 0/1 mask for diagonal blocks of S^T
    mask_np = (np.arange(128)[:, None] < np.arange(128)[None, :]).astype(mm_np)
    maskd = nc.inline_tensor(np.ascontiguousarray(mask_np), name="mask").ap()

    with tile.TileContext(nc) as tc, ExitStack() as ctx:
        wide = MM_MODE == "bf16"  # f32-storage modes need smaller pools to fit SBUF
        const = ctx.enter_context(tc.tile_pool(name="const", bufs=1))
        xpool = ctx.enter_context(tc.tile_pool(name="xpool", bufs=2 if wide else 1))
        qkpool = ctx.enter_context(tc.tile_pool(name="qkpool", bufs=6))
        vpool = ctx.enter_context(tc.tile_pool(name="vpool", bufs=2))
        ptpool = ctx.enter_context(tc.tile_pool(name="ptpool", bufs=34 if wide else 12))
        small = ctx.enter_context(tc.tile_pool(name="small", bufs=4 if wide else 2))
        psAcc = ctx.enter_context(tc.tile_pool(name="psAcc", bufs=2, space="PSUM"))
        psS = ctx.enter_context(tc.tile_pool(name="psS", bufs=3, space="PSUM"))

        # --- constants to SBUF ---
        w_sb = {}
        for name, drt in (("wq", wqt), ("wk", wkt), ("wv", wvt)):
            t = const.tile([128, 4, C], mm_dt, tag=f"w_{name}", name=f"w_{name}")
            nc.sync.dma_start(t, drt.rearrange("(ko p) m -> p ko m", p=128))
            w_sb[name] = t
        bq_sb = const.tile([128, 4], f32)
        nc.sync.dma_start(bq_sb, bqp)
        bk_sb = const.tile([128, 4], f32)
        nc.sync.dma_start(bk_sb, bkp)
        bv_sb = const.tile([1, C], f32)
        nc.sync.dma_start(bv_sb, bvr)
        mask_sb = const.tile([128, 128], mm_dt)
        nc.sync.dma_start(mask_sb, maskd)
        ones_row = const.tile([1, 128], f32)
        nc.vector.memset(ones_row, 1.0)

        qt_all, kt_all, v_all = {}, {}, {}
        x_all = {}

        def load_x(b):
            xq_sb = xpool.tile([128, 4, S], mm_dt, tag="xq", name=f"xq_{b}")
            xk_sb = xpool.tile([128, 4, S], mm_dt, tag="xk", name=f"xk_{b}")
            for kc in range(4):
                nc.sync.dma_start(
                    xk_sb[:, kc], xk[b].rearrange("(ko p) s -> p ko s", p=128)[:, kc])
                nc.sync.dma_start(
                    xq_sb[:, kc], xq[b].rearrange("(ko p) s -> p ko s", p=128)[:, kc])
            x_all[b] = (xq_sb, xk_sb)

        def v_gen(b):
            # V projection: [seq, ch] layout, 65-strided per head with ones col
            xq_sb, xk_sb = x_all[b]
            v_sb = vpool.tile([128, KB, NH * 65], mm_dt, tag="v", name=f"v_{b}")
            v_view = v_sb.rearrange("p k (h c) -> p k h c", c=65)
            nc.vector.memset(v_view[:, :, :, 64:65], 1.0)
            v_all[b] = v_sb
            for kb in range(KB):
                ps = psAcc.tile([128, 512], f32, tag="acc")
                first = True
                if has_bv:
                    nc.tensor.matmul(ps, lhsT=ones_row, rhs=bv_sb,
                                     start=True, stop=False)
                    first = False
                for kc in range(4):
                    nc.tensor.matmul(
                        ps,
                        lhsT=_mm_cast(xk_sb[:, kc, kb * 128:(kb + 1) * 128]),
                        rhs=_mm_cast(w_sb["wv"][:, kc, :]),
                        start=first,
                        stop=(kc == 3),
                    )
                    first = False
                nc.vector.tensor_copy(
                    out=v_view[:, kb, :, 0:64],
                    in_=ps.rearrange("p (h c) -> p h c", c=64),
                )
                yield

        def qk_gen(b, hp):
            # Q^T / K^T projection slices for one head pair: [128ch, seq]
            xq_sb, xk_sb = x_all[b]
            qt_sb = qkpool.tile([128, S], mm_dt, tag="qt", name=f"qt_{b}_{hp}")
            kt_sb = qkpool.tile([128, S], mm_dt, tag="kt", name=f"kt_{b}_{hp}")
            qt_all[(b, hp)], kt_all[(b, hp)] = qt_sb, kt_sb
            mc = hp
            for src, wname, bias_sb, dst in (
                (xk_sb, "wk", bk_sb, kt_sb),
                (xq_sb, "wq", bq_sb, qt_sb),
            ):
                for sc in range(0, S, 512):
                    ps = psAcc.tile([128, 512], f32, tag="acc")
                    for kc in range(4):
                        nc.tensor.matmul(
                            ps,
                            lhsT=_mm_cast(w_sb[wname][:, kc, mc * 128:(mc + 1) * 128]),
                            rhs=_mm_cast(src[:, kc, sc:sc + 512]),
                            start=(kc == 0),
                            stop=(kc == 3),
                        )
                    nc.vector.tensor_scalar_add(
                        dst[:, sc:sc + 512], ps, bias_sb[:, mc:mc + 1]
                    )
                    yield

        def attn_hp_gen(b, hp):
            qt_sb, kt_sb, v_sb = qt_all[(b, hp)], kt_all[(b, hp)], v_all[b]
            # phase 1: per k-block, scores over the remaining q-range [128kb, 1024)
            # into a 2-bank PSUM tile; causal mask added on PE; ONE exp per (kb, head).
            pts = {}
            for kb in range(KB):
                q0 = kb * 128
                ncols = S - q0
                for hh in range(2):
                    pb = hh * 64
                    s_ps = psS.tile([128, 1024], f32, tag="s",
                                    name=f"s_{b}_{hp}_{kb}_{hh}")
                    for c in range(0, ncols, 512):
                        n = min(512, ncols - c)
                        nc.tensor.matmul(
                            s_ps[:, c:c + n],
                            lhsT=_mm_cast(kt_sb[pb:pb + 64, kb * 128:(kb + 1) * 128]),
                            rhs=_mm_cast(qt_sb[pb:pb + 64, q0 + c:q0 + c + n]),
                            start=True,
                            stop=True,
                        )
                    p_sb = ptpool.tile([128, 1024], mm_dt, tag="pt",
                                       name=f"pt_{b}_{hp}_{kb}_{hh}")
                    for c in range(0, ncols, 512):
                        n = min(512, ncols - c)
                        nc.scalar.activation(
                            p_sb[:, c:c + n], s_ps[:, c:c + n],
                            mybir.ActivationFunctionType.Exp, scale=0.125,
                        )
                    # diagonal block (first 128 cols): strictly-upper mask
                    nc.vector.tensor_mul(p_sb[:, 0:128], p_sb[:, 0:128], mask_sb)
                    pts[(kb, hh)] = p_sb
                    if hh == 1:
                        yield
            # phase 2: AV matmul bursts per q-chunk
            for qc in range(QCN):
                kmax = 4 * (qc + 1)
                o_ps = [psAcc.tile([128, 512], f32, tag="acc",
                                   name=f"o_{b}_{hp}_{qc}_{i}")
                        for i in range(2)]
                for kb in range(kmax):
                    q0 = max(kb * 128, qc * 512)
                    n = qc * 512 + 512 - q0
                    c0 = q0 - qc * 512          # offset in o_ps
                    p0 = q0 - kb * 128          # offset in pt tile
                    for hh in range(2):
                        h = hp * 2 + hh
                        nc.tensor.matmul(
                            o_ps[hh][0:65, c0:c0 + n],
                            lhsT=_mm_cast(v_sb[:, kb, h * 65:h * 65 + 65]),
                            rhs=_mm_cast(pts[(kb, hh)][:, p0:p0 + n]),
                            start=(kb == 0),
                            stop=(kb == kmax - 1),
                        )
                for hh in range(2):
                    h = hp * 2 + hh
                    o_sb = small.tile([65, 512], f32, tag="osb")
                    nc.vector.tensor_copy(o_sb, o_ps[hh][0:65, :])
                    nc.sync.dma_start(out[b, h, qc], o_sb)
                    yield

        _SENT = object()

        def drain(g):
            for _ in g:
                pass

        # prologue: inputs for b0, V(b0), QK(b0, hp0) so attention can start
        load_x(0)
        drain(v_gen(0))
        drain(qk_gen(0, 0))
        load_x(1)
        # pending proj groups, in dependency-safe order, to be spread across
        # the attention stream (each unit's deps are >=1 unit ahead)
        pending = _chain([qk_gen(0, 1), qk_gen(0, 2), qk_gen(0, 3),
                          v_gen(1), qk_gen(1, 0), qk_gen(1, 1),
                          qk_gen(1, 2), qk_gen(1, 3)])
        units = [(0, 1), (0, 2), (0, 3)] + [(1, hp) for hp in range(4)]
        # 36 proj groups over 8 units -> ~5 per unit, front-loaded
        attn_units = [(0, 0)] + units
        for b, hp in attn_units:
            ag = attn_hp_gen(b, hp)
            done_a = False
            while not done_a:
                done_a = next(ag, _SENT) is _SENT
                next(pending, None)
        drain(pending)

    nc.compile()
    return nc


def postprocess(oraw):
    """oraw [BL, NH, QCN, 65, 512] -> normalized [BL, C, S].
    Row 64 carries the attention row-sums; q=0 has sum 0 (start_mask row)."""
    o = oraw[:, :, :, :64, :]                  # [BL, NH, QCN, 64, 512]
    s = oraw[:, :, :, 64:65, :]                # [BL, NH, QCN, 1, 512]
    with np.errstate(divide="ignore", invalid="ignore"):
        n = np.where(s > 0, o / s, 0.0).astype(np.float32)
    # [BL, NH, QCN, 64, 512] -> [BL, NH, 64, QCN, 512] -> [BL, C, S]
    return np.ascontiguousarray(n.transpose(0, 1, 3, 2, 4)).reshape(BL, C, S)


def _wn_t(v, g):
    # weight-norm (matches reference wn_weight), returned transposed [in, out]
    norm = np.sqrt(np.sum(v * v, axis=1, keepdims=True))
    wt = (v * (g[:, None] / norm)).T
    return np.ascontiguousarray(wt)


def prepare_in_maps(inputs):
    mm_np = _mm_np()
    query = np.asarray(inputs["query"], np.float32).reshape(B, CQ, S)
    key = np.asarray(inputs["key"], np.float32).reshape(B, CK, S)
    wqt = _wn_t(np.asarray(inputs["vq"], np.float32), np.asarray(inputs["gq"], np.float32)).astype(mm_np)
    wkt = _wn_t(np.asarray(inputs["vk"], np.float32), np.asarray(inputs["gk"], np.float32)).astype(mm_np)
    wvt = _wn_t(np.asarray(inputs["vv"], np.float32), np.asarray(inputs["gv"], np.float32)).astype(mm_np)
    bq = np.asarray(inputs["bq"], np.float32)
    bk = np.asarray(inputs["bk"], np.float32)
    bv = np.asarray(inputs["bv"], np.float32)
    bqp = np.ascontiguousarray(bq.reshape(4, 128).T)
    bkp = np.ascontiguousarray(bk.reshape(4, 128).T)
    bvr = np.ascontiguousarray(bv.reshape(1, C))

    in_maps = []
    for i in range(NCORES):
        in_maps.append({
            "xq": np.ascontiguousarray(query[i * BL:(i + 1) * BL]).astype(mm_np),
            "xk": np.ascontiguousarray(key[i * BL:(i + 1) * BL]).astype(mm_np),
            "wqt": wqt, "wkt": wkt, "wvt": wvt,
            "bqp": bqp, "bkp": bkp, "bvr": bvr,
        })
    has_bv = bool(np.any(bv))
    return in_maps, has_bv


_NC_CACHE = {}


def run(inputs, trace=False):
    in_maps, has_bv = prepare_in_maps(inputs)
    cache_key = (MM_MODE, has_bv)
    if cache_key not in _NC_CACHE:
        _NC_CACHE[cache_key] = build_nc(has_bv)
    nc = _NC_CACHE[cache_key]
    try:
        res = run_bass_kernel_spmd(nc, in_maps, core_ids=list(range(NCORES)),
                                   trace=trace)
    except ModuleNotFoundError:
        # axon NTFF profile hook unavailable in this environment
        res = run_bass_kernel_spmd(nc, in_maps, core_ids=list(range(NCORES)),
                                   trace=False)
    outs = [postprocess(r["out"]) for r in res.results]
    full = np.concatenate(outs, axis=0).reshape(B, CH, H, W)
    return full, res


def kernel(**inputs) -> np.ndarray:
    out, _ = run(inputs, trace=False)
    return out


# revision 26
# speedup vs baseline: 1.0882x; 1.0882x over previous
"""Causal attention (weight-normed QKV proj + strictly-causal softmax) on 8 trn2 cores.

Sharding: data-parallel over batch (16 batches -> 2 per core). Each core runs an
identical NEFF (SPMD, no collectives) computing its 2 batches end-to-end.

Device-side formulation is fully "transposed" so no on-device transposes are needed:
  inputs are channels-first [C, S]  (S = H*W = 1024 spatial positions)
  Q^T, K^T computed as [C_head, S] via matmul(lhsT=W_T chunk, rhs=X)
  V computed as [S, C] via matmul(lhsT=X chunk, rhs=W_T)  (+ an appended ones
  column per head so the attention row-sums fall out of the AV matmul for free)
  S^T[k,q] = K_blk @ Q^T   (contract d=64; two heads packed in the PE array via
  partition offsets 0/64)
  P^T = exp(S^T/8) (no row-max subtraction needed: |S/8| <= ~30), with a
  strictly-upper 0/1 mask multiplied onto each diagonal block
  O^T[d,q] (+ sums row 64) = V_ext accumulate over k-blocks, causal blocks skipped
  The final row-sum division (0.07% of FLOPs) happens on the host in
  postprocess(); the q=0 column (start_mask) falls out as 0/0 -> 0 there.
  Projection work is interleaved into the attention stream at emission time to
  keep the PE's HAM clock-gate warm.
"""

import os
from contextlib import ExitStack
from itertools import chain as _chain_mod

import ml_dtypes
import numpy as np

import concourse.bass as bass
import concourse.mybir as mybir
import concourse.tile as tile
from concourse import bacc
from concourse.bass_utils import run_bass_kernel_spmd

# Problem constants (hardcoded per contest contract).
B, CQ, CK, CH, NH, H, W = 16, 512, 512, 512, 8, 32, 32
S = H * W            # 1024
DH = CH // NH        # 64
NCORES = 8
BL = B // NCORES     # 2 batches per core
C = 512
KB = S // 128        # 8 k-blocks
QCN = S // 512       # 2 q-chunks of 512

f32 = mybir.dt.float32

# Matmul dtype knob: "bf16" | "f32" | "f32r"
MM_MODE = os.environ.get("KERNEL_MM_MODE", "bf16")


def _mm_dt():
    return {"bf16": mybir.dt.bfloat16, "f32": f32, "f32r": f32}[MM_MODE]


def _mm_np():
    return {"bf16": ml_dtypes.bfloat16, "f32": np.float32, "f32r": np.float32}[MM_MODE]


def _mm_cast(ap):
    # view an f32 AP as f32r for matmul operands in f32r mode
    if MM_MODE == "f32r":
        return ap.bitcast(mybir.dt.float32r)
    return ap


def _chain(gens):
    for g in gens:
        yield from g


def build_nc(has_bv: bool):
    mm_dt = _mm_dt()
    mm_np = _mm_np()
    nc = bacc.Bacc("TRN2", target_bir_lowering=False, debug=False,
                   num_devices=NCORES)

    xq = nc.dram_tensor("xq", [BL, C, S], mm_dt, kind="ExternalInput").ap()
    xk = nc.dram_tensor("xk", [BL, C, S], mm_dt, kind="ExternalInput").ap()
    wqt = nc.dram_tensor("wqt", [C, C], mm_dt, kind="ExternalInput").ap()
    wkt = nc.dram_tensor("wkt", [C, C], mm_dt, kind="ExternalInput").ap()
    wvt = nc.dram_tensor("wvt", [C, C], mm_dt, kind="ExternalInput").ap()
    bqp = nc.dram_tensor("bqp", [128, 4], f32, kind="ExternalInput").ap()
    bkp = nc.dram_tensor("bkp", [128, 4], f32, kind="ExternalInput").ap()
    bvr = nc.dram_tensor("bvr", [1, C], f32, kind="ExternalInput").ap()
    out = nc.dram_tensor("out", [BL, NH, QCN, 65, 512], f32, kind="ExternalOutput").ap()

    # strictly-upper-triangular 0/1 mask for diagonal blocks of S^T
    mask_np = (np.arange(128)[:, None] < np.arange(128)[None, :]).astype(mm_np)
    maskd = nc.inline_tensor(np.ascontiguousarray(mask_np), name="mask").ap()

    with tile.TileContext(nc) as tc, ExitStack() as ctx:
        wide = MM_MODE == "bf16"  # f32-storage modes need smaller pools to fit SBUF
        const = ctx.enter_context(tc.tile_pool(name="const", bufs=1))
        xpool = ctx.enter_context(tc.tile_pool(name="xpool", bufs=2 if wide else 1))
        qkpool = ctx.enter_context(tc.tile_pool(name="qkpool", bufs=6))
        vpool = ctx.enter_context(tc.tile_pool(name="vpool", bufs=2))
        ptpool = ctx.enter_context(tc.tile_pool(name="ptpool", bufs=34 if wide else 12))
        small = ctx.enter_context(tc.tile_pool(name="small", bufs=4 if wide else 2))
        psAcc = ctx.enter_context(tc.tile_pool(name="psAcc", bufs=2, space="PSUM"))
        psS = ctx.enter_context(tc.tile_pool(name="psS", bufs=3, space="PSUM"))

        # --- constants to SBUF ---
        w_sb = {}
        for name, drt in (("wq", wqt), ("wk", wkt), ("wv", wvt)):
            t = const.tile([128, 4, C], mm_dt, tag=f"w_{name}", name=f"w_{name}")
            nc.sync.dma_start(t, drt.rearrange("(ko p) m -> p ko m", p=128))
            w_sb[name] = t
        bq_sb = const.tile([128, 4], f32)
        nc.sync.dma_start(bq_sb, bqp)
        bk_sb = const.tile([128, 4], f32)
        nc.sync.dma_start(bk_sb, bkp)
        bv_sb = const.tile([1, C], f32)
        nc.sync.dma_start(bv_sb, bvr)
        mask_sb = const.tile([128, 128], mm_dt)
        nc.sync.dma_start(mask_sb, maskd)
        ones_row = const.tile([1, 128], f32)
        nc.vector.memset(ones_row, 1.0)

        qt_all, kt_all, v_all = {}, {}, {}
        x_all = {}

        def load_x(b):
            xq_sb = xpool.tile([128, 4, S], mm_dt, tag="xq", name=f"xq_{b}")
            xk_sb = xpool.tile([128, 4, S], mm_dt, tag="xk", name=f"xk_{b}")
            for kc in range(4):
                nc.sync.dma_start(
                    xk_sb[:, kc], xk[b].rearrange("(ko p) s -> p ko s", p=128)[:, kc])
                nc.sync.dma_start(
                    xq_sb[:, kc], xq[b].rearrange("(ko p) s -> p ko s", p=128)[:, kc])
            x_all[b] = (xq_sb, xk_sb)

        def v_gen(b):
            # V projection: [seq, ch] layout, 65-strided per head with ones col
            xq_sb, xk_sb = x_all[b]
            v_sb = vpool.tile([128, KB, NH * 65], mm_dt, tag="v", name=f"v_{b}")
            v_view = v_sb.rearrange("p k (h c) -> p k h c", c=65)
            nc.vector.memset(v_view[:, :, :, 64:65], 1.0)
            v_all[b] = v_sb
            for kb in range(KB):
                ps = psAcc.tile([128, 512], f32, tag="acc")
                first = True
                if has_bv:
                    nc.tensor.matmul(ps, lhsT=ones_row, rhs=bv_sb,
                                     start=True, stop=False)
                    first = False
                for kc in range(4):
                    nc.tensor.matmul(
                        ps,
                        lhsT=_mm_cast(xk_sb[:, kc, kb * 128:(kb + 1) * 128]),
                        rhs=_mm_cast(w_sb["wv"][:, kc, :]),
                        start=first,
                        stop=(kc == 3),
                    )
                    first = False
                nc.vector.tensor_copy(
                    out=v_view[:, kb, :, 0:64],
                    in_=ps.rearrange("p (h c) -> p h c", c=64),
                )
                yield

        def qk_gen(b, hp):
            # Q^T / K^T projection slices for one head pair: [128ch, seq]
            xq_sb, xk_sb = x_all[b]
            qt_sb = qkpool.tile([128, S], mm_dt, tag="qt", name=f"qt_{b}_{hp}")
            kt_sb = qkpool.tile([128, S], mm_dt, tag="kt", name=f"kt_{b}_{hp}")
            qt_all[(b, hp)], kt_all[(b, hp)] = qt_sb, kt_sb
            mc = hp
            for src, wname, bias_sb, dst in (
                (xk_sb, "wk", bk_sb, kt_sb),
                (xq_sb, "wq", bq_sb, qt_sb),
            ):
                for sc in range(0, S, 512):
                    ps = psAcc.tile([128, 512], f32, tag="acc")
                    for kc in range(4):
                        nc.tensor.matmul(
                            ps,
                            lhsT=_mm_cast(w_sb[wname][:, kc, mc * 128:(mc + 1) * 128]),
                            rhs=_mm_cast(src[:, kc, sc:sc + 512]),
                            start=(kc == 0),
                            stop=(kc == 3),
                        )
                    nc.vector.tensor_scalar_add(
                        dst[:, sc:sc + 512], ps, bias_sb[:, mc:mc + 1]
                    )
                    yield

        def attn_hp_gen(b, hp):
            qt_sb, kt_sb, v_sb = qt_all[(b, hp)], kt_all[(b, hp)], v_all[b]
            # phase 1: per k-block, scores over the remaining q-range [128kb, 1024)
            # into a 2-bank PSUM tile; causal mask added on PE; ONE exp per (kb, head).
            pts = {}
            for kb in range(KB):
                q0 = kb * 128
                ncols = S - q0
                s_tiles = [psS.tile([128, 1024], f32, tag="s",
                                    name=f"s_{b}_{hp}_{kb}_{i}")
                           for i in range(2)]
                for c in range(0, ncols, 512):
                    n = min(512, ncols - c)
                    for hh in range(2):
                        pb = hh * 64
                        nc.tensor.matmul(
                            s_tiles[hh][:, c:c + n],
                            lhsT=_mm_cast(kt_sb[pb:pb + 64, kb * 128:(kb + 1) * 128]),
                            rhs=_mm_cast(qt_sb[pb:pb + 64, q0 + c:q0 + c + n]),
                            start=True,
                            stop=True,
                        )
                for hh in range(2):
                    s_ps = s_tiles[hh]
                    p_sb = ptpool.tile([128, 1024], mm_dt, tag="pt",
                                       name=f"pt_{b}_{hp}_{kb}_{hh}")
                    for c in range(0, ncols, 512):
                        n = min(512, ncols - c)
                        nc.scalar.activation(
                            p_sb[:, c:c + n], s_ps[:, c:c + n],
                            mybir.ActivationFunctionType.Exp, scale=0.125,
                        )
                    # diagonal block (first 128 cols): strictly-upper mask
                    nc.vector.tensor_mul(p_sb[:, 0:128], p_sb[:, 0:128], mask_sb)
                    pts[(kb, hh)] = p_sb
                    if hh == 1:
                        yield
            # phase 2: AV matmul bursts per q-chunk
            for qc in range(QCN):
                kmax = 4 * (qc + 1)
                o_ps = [psAcc.tile([128, 512], f32, tag="acc",
                                   name=f"o_{b}_{hp}_{qc}_{i}")
                        for i in range(2)]
                for kb in range(kmax):
                    q0 = max(kb * 128, qc * 512)
                    n = qc * 512 + 512 - q0
                    c0 = q0 - qc * 512          # offset in o_ps
                    p0 = q0 - kb * 128          # offset in pt tile
                    for hh in range(2):
                        h = hp * 2 + hh
                        nc.tensor.matmul(
                            o_ps[hh][0:65, c0:c0 + n],
                            lhsT=_mm_cast(v_sb[:, kb, h * 65:h * 65 + 65]),
                            rhs=_mm_cast(pts[(kb, hh)][:, p0:p0 + n]),
                            start=(kb == 0),
                            stop=(kb == kmax - 1),
                        )
                for hh in range(2):
                    h = hp * 2 + hh
                    o_sb = small.tile([65, 512], f32, tag="osb")
                    nc.vector.tensor_copy(o_sb, o_ps[hh][0:65, :])
                    nc.sync.dma_start(out[b, h, qc], o_sb)
                    yield

        _SENT = object()

        def drain(g):
            for _ in g:
                pass

        # prologue: inputs for b0, V(b0), QK(b0, hp0) so attention can start
        load_x(0)
        drain(v_gen(0))
        drain(qk_gen(0, 0))
        load_x(1)
        # pending proj groups, in dependency-safe order, to be spread across
        # the attention stream (each unit's deps are >=1 unit ahead)
        pending = _chain([qk_gen(0, 1), qk_gen(0, 2), qk_gen(0, 3),
                          v_gen(1), qk_gen(1, 0), qk_gen(1, 1),
                          qk_gen(1, 2), qk_gen(1, 3)])
        units = [(0, 1), (0, 2), (0, 3)] + [(1, hp) for hp in range(4)]
        # 36 proj groups over 8 units -> ~5 per unit, front-loaded
        attn_units = [(0, 0)] + units
        tick = 0
        for b, hp in attn_units:
            ag = attn_hp_gen(b, hp)
            done_a = False
            while not done_a:
                done_a = next(ag, _SENT) is _SENT
                tick += 1
                if tick % 2 == 0:
                    next(pending, None)
        drain(pending)

    nc.compile()
    return nc


def postprocess(oraw):
    """oraw [BL, NH, QCN, 65, 512] -> normalized [BL, C, S].
    Row 64 carries the attention row-sums; q=0 has sum 0 (start_mask row)."""
    o = oraw[:, :, :, :64, :]                  # [BL, NH, QCN, 64, 512]
    s = oraw[:, :, :, 64:65, :]                # [BL, NH, QCN, 1, 512]
    with np.errstate(divide="ignore", invalid="ignore"):
        n = np.where(s > 0, o / s, 0.0).astype(np.float32)
    # [BL, NH, QCN, 64, 512] -> [BL, NH, 64, QCN, 512] -> [BL, C, S]
    return np.ascontiguousarray(n.transpose(0, 1, 3, 2, 4)).reshape(BL, C, S)


def _wn_t(v, g):
    # weight-norm (matches reference wn_weight), returned transposed [in, out]
    norm = np.sqrt(np.sum(v * v, axis=1, keepdims=True))
    wt = (v * (g[:, None] / norm)).T
    return np.ascontiguousarray(wt)


def prepare_in_maps(inputs):
    mm_np = _mm_np()
    query = np.asarray(inputs["query"], np.float32).reshape(B, CQ, S)
    key = np.asarray(inputs["key"], np.float32).reshape(B, CK, S)
    wqt = _wn_t(np.asarray(inputs["vq"], np.float32), np.asarray(inputs["gq"], np.float32)).astype(mm_np)
    wkt = _wn_t(np.asarray(inputs["vk"], np.float32), np.asarray(inputs["gk"], np.float32)).astype(mm_np)
    wvt = _wn_t(np.asarray(inputs["vv"], np.float32), np.asarray(inputs["gv"], np.float32)).astype(mm_np)
    bq = np.asarray(inputs["bq"], np.float32)
    bk = np.asarray(inputs["bk"], np.float32)
    bv = np.asarray(inputs["bv"], np.float32)
    bqp = np.ascontiguousarray(bq.reshape(4, 128).T)
    bkp = np.ascontiguousarray(bk.reshape(4, 128).T)
    bvr = np.ascontiguousarray(bv.reshape(1, C))

    in_maps = []
    for i in range(NCORES):
        in_maps.append({
            "xq": np.ascontiguousarray(query[i * BL:(i + 1) * BL]).astype(mm_np),
            "xk": np.ascontiguousarray(key[i * BL:(i + 1) * BL]).astype(mm_np),
            "wqt": wqt, "wkt": wkt, "wvt": wvt,
            "bqp": bqp, "bkp": bkp, "bvr": bvr,
        })
    has_bv = bool(np.any(bv))
    return in_maps, has_bv


_NC_CACHE = {}


def run(inputs, trace=False):
    in_maps, has_bv = prepare_in_maps(inputs)
    cache_key = (MM_MODE, has_bv)
    if cache_key not in _NC_CACHE:
        _NC_CACHE[cache_key] = build_nc(has_bv)
    nc = _NC_CACHE[cache_key]
    try:
        res = run_bass_kernel_spmd(nc, in_maps, core_ids=list(range(NCORES)),
                                   trace=trace)
    except ModuleNotFoundError:
        # axon NTFF profile hook unavailable in this environment
        res = run_bass_kernel_spmd(nc, in_maps, core_ids=list(range(NCORES)),
                                   trace=False)
    outs = [postprocess(r["out"]) for r in res.results]
    full = np.concatenate(outs, axis=0).reshape(B, CH, H, W)
    return full, res


def kernel(**inputs) -> np.ndarray:
    out, _ = run(inputs, trace=False)
    return out
